# revision 42
# baseline (speedup 1.0000x reference)
"""Self-contained Trainium2 kernel for nn_MultiHeadAttention_91070486544496.

B=4, S=2048, D=1024, H=16 causal MHA. 8-core SPMD, head-parallel:
each core computes QKV+attention for its 2 heads over all batches, then
multiplies its context slice by the matching WO rows and writes a full
[BS, D] partial output; the host sums the 8 partials. No collectives.

Schedule keeps the TensorEngine gap-free (DVFS ramps 1.2->2.4GHz after
3us of continuous busy): ctx matmuls trail the score stream via a
deferred-work queue, V-transposes are deferred one chunk, softmax
normalization (reciprocal on DVE, broadcast matmul, multiply on Pool)
trails by a full head-stage, and output-projection matmuls are injected
as PE filler between score pairs. Causal masking is done by trimming
fully-masked column bands (never computed) and zeroing the partially
masked 128-wide diagonal band of exp(scores) with a Pool affine_select.
"""
import sys

for _p in ("/opt/trn_rl_repo", "/root/.axon_site/_ro/trn_rl_repo"):
    if _p not in sys.path:
        sys.path.append(_p)

import numpy as np

# ======== runtime infra (axon NTFF hook, BIR wait splitter) ========

import contextlib
import ctypes
import json
import types

_SO_PATH = "/opt/axon/libaxon_pjrt.so"


def _ntff_profile_via_ctypes(so_path):
    lib = ctypes.CDLL(so_path)
    if not hasattr(lib, "axon_start_nrt_profile"):
        return None
    lib.axon_start_nrt_profile.argtypes = [
        ctypes.POINTER(ctypes.c_int64),
        ctypes.c_size_t,
    ]
    lib.axon_start_nrt_profile.restype = ctypes.c_int64
    lib.axon_stop_nrt_profile.argtypes = [ctypes.c_char_p]
    lib.axon_stop_nrt_profile.restype = ctypes.c_int64

    @contextlib.contextmanager
    def _hook(output_dir, device_ids):
        import jax
        jax.devices()
        if device_ids:
            ids = (ctypes.c_int64 * len(device_ids))(*device_ids)
            rc = lib.axon_start_nrt_profile(ids, len(device_ids))
        else:
            rc = lib.axon_start_nrt_profile(None, 0)
        if rc != 0:
            raise RuntimeError(f"axon_start_nrt_profile rc={rc}")
        try:
            yield
        finally:
            n = lib.axon_stop_nrt_profile(str(output_dir).encode())
            if n < 0:
                raise RuntimeError(f"axon_stop_nrt_profile rc={n}")

    return _hook


def split_multi_waits(bir_json: bytes) -> bytes:
    d = json.loads(bir_json)
    n_split = 0
    for fn in d.get("functions", []):
        for blk in fn.get("blocks", []):
            insts = blk.get("instructions", [])
            out = []
            for inst in insts:
                si = inst.get("sync_info")
                waits = (si or {}).get("on_wait") or []
                if len(waits) > 1:
                    extra, keep = waits[:-1], waits[-1:]
                    for k, w in enumerate(extra):
                        out.append({
                            "debug": inst.get("debug", 0),
                            "engine": inst["engine"],
                            "ins": [],
                            "outs": [],
                            "name": f"{inst['name']}-ws{k}",
                            "opcode": "NoOp",
                            "sync_info": {"on_update": [], "on_wait": [w]},
                        })
                        n_split += 1
                    si["on_wait"] = keep
                out.append(inst)
            blk["instructions"] = out
    if n_split:
        print(f"bass_infra: split {n_split} extra sync waits into NoOps")
    return json.dumps(d).encode()


def install():
    # 1. antenv.axon_hooks shim
    if "antenv.axon_hooks" not in sys.modules:
        mod = types.ModuleType("antenv.axon_hooks")
        _state = {"hook": _ntff_profile_via_ctypes(_SO_PATH)}
        mod.set_axon_ntff_profile_hook = lambda h: _state.__setitem__("hook", h)
        mod.get_axon_ntff_profile_hook = lambda: _state["hook"]
        sys.modules["antenv.axon_hooks"] = mod
        import antenv
        antenv.axon_hooks = mod

    from concourse import bass_utils, bass2jax

    # 2. upload_artifacts stub
    bass_utils.upload_artifacts = lambda tmpdir: tmpdir

    # 3. wait-splitting compile wrapper
    orig_compile = bass_utils.compile_bir_kernel

    def compile_with_split(bir_json, tmpdir, neff_name="file.neff"):
        return orig_compile(split_multi_waits(bir_json), tmpdir, neff_name=neff_name)

    if getattr(bass2jax.compile_bir_kernel, "__name__", "") != "compile_with_split":
        bass_utils.compile_bir_kernel = compile_with_split
        bass2jax.compile_bir_kernel = compile_with_split


# ======== kernel IR builder ========
from collections import deque
from contextlib import ExitStack

import concourse.bass as bass
import concourse.mybir as mybir
import concourse.tile as tile
from concourse.bass import ds, ts
from concourse.masks import make_identity

F32 = mybir.dt.float32
BF16 = mybir.dt.bfloat16
EXP = mybir.ActivationFunctionType.Exp
LN = mybir.ActivationFunctionType.Ln

B, S, D, H, DK = 4, 2048, 1024, 16, 64
NC = 8          # cores
HL = 2          # heads per core
BS = B * S      # 8192
NQ = S // 512   # q-chunks per batch = 4
NKC = S // 128  # k-chunks per batch = 16
NDC = D // 128  # d_in chunks = 8
TRAIL = 7       # deferred-work queue depth (ki items)


def build(cfg=None):
    nc = bass.Bass("TRN2", target_bir_lowering=False, debug=False, num_devices=NC)

    xT = nc.dram_tensor("xT", [D, BS], BF16, kind="ExternalInput")
    wq = nc.dram_tensor("wq", [D, 128], BF16, kind="ExternalInput")
    wk = nc.dram_tensor("wk", [D, 128], BF16, kind="ExternalInput")
    wv = nc.dram_tensor("wv", [D, 128], BF16, kind="ExternalInput")
    wo = nc.dram_tensor("wo", [128, D], BF16, kind="ExternalInput")
    out = nc.dram_tensor("out", [BS, D], BF16, kind="ExternalOutput")

    with tile.TileContext(nc) as tc, ExitStack() as ctx:
        const = ctx.enter_context(tc.tile_pool(name="const", bufs=1))
        wpool = ctx.enter_context(tc.tile_pool(name="wpool", bufs=1))
        xpool = ctx.enter_context(tc.tile_pool(name="xpool", bufs=3))
        qkv_ps = ctx.enter_context(tc.tile_pool(name="qkv_ps", bufs=2, space="PSUM"))
        sp_ps = ctx.enter_context(tc.tile_pool(name="sp_ps", bufs=2, space="PSUM"))
        cp_ps = ctx.enter_context(tc.tile_pool(name="cp_ps", bufs=2, space="PSUM"))
        qk_sb = ctx.enter_context(tc.tile_pool(name="qk_sb", bufs=1))
        vpool = ctx.enter_context(tc.tile_pool(name="vpool", bufs=2))
        et_sb = ctx.enter_context(tc.tile_pool(name="et_sb", bufs=12))
        ep_sb = ctx.enter_context(tc.tile_pool(name="ep_sb", bufs=2))
        ctx_sb = ctx.enter_context(tc.tile_pool(name="ctx_sb", bufs=2))
        ob_sb = ctx.enter_context(tc.tile_pool(name="ob_sb", bufs=4))

        # ---- constants ----
        ident = const.tile([128, 128], F32)
        make_identity(nc, ident[:])
        identr = const.tile([128, 128], BF16)
        nc.vector.tensor_copy(identr[:], ident[:])
        onesf = const.tile([128, 16], F32)
        nc.vector.memset(onesf[:], 1.0)
        ones_l = const.tile([1, 64], F32)
        nc.vector.memset(ones_l[:], 1.0)
        ones_lr = const.tile([1, 64], BF16)
        nc.vector.tensor_copy(ones_lr[:], ones_l[:])
        ones_row = const.tile([1, 512], F32)
        nc.vector.memset(ones_row[:], 1.0)

        # ---- weights ----
        wq_sb = wpool.tile([128, NDC, 128], BF16)
        wk_sb = wpool.tile([128, NDC, 128], BF16)
        wv_sb = wpool.tile([128, NDC, 128], BF16)
        wo_sb = wpool.tile([128, D], BF16)
        nc.sync.dma_start(wq_sb[:], wq.rearrange("(j p) h -> p j h", p=128))

        # ---- deferred work ----
        ctxq = deque()   # trailing ctx matmuls + softmax epilogue parts
        aux = deque()    # out-projection bundles (PE filler)
        state = {"partB": None, "slot": 0}

        def pump_ctx(target):
            while len(ctxq) > target:
                ctxq.popleft()()

        def pump_aux(n=1, force=False):
            for _ in range(n):
                if not aux:
                    return
                if not force and aux[0][0] > state["slot"]:
                    return
                aux.popleft()[1]()

        def tick():
            state["slot"] += 1
            if state["slot"] % 3 == 0:
                pump_aux(1)

        # ---- x prefetch ----
        xt_tiles = {}

        def emit_x_dma(b, i, split=False):
            if b >= B or (b, i) in xt_tiles:
                return
            t = xpool.tile([128, NDC, 512], BF16, tag="xt", name=f"xt{b}_{i}")
            xt_tiles[(b, i)] = t
            src = xT.rearrange("(j p) n -> p j n", p=128)[:, :, ds(b * S + i * 512, 512)]
            if split:
                for j in range(NDC):
                    nc.sync.dma_start(t[:, j, :], src[:, j, :])
            else:
                nc.sync.dma_start(t[:], src)

        # ---- out-projection bundle: one PE matmul + Pool copy + DMA ----
        def make_outproj(b, qi, c, nn, ctxT):
            def run():
                op = qkv_ps.tile([128, 512], F32, tag="qkv", name="op")
                nc.tensor.matmul(op[:], ctxT[:, ds(qi * 512 + c * 128, 128)],
                                 wo_sb[:, ds(nn * 512, 512)], start=True, stop=True)
                ob = ob_sb.tile([128, 512], BF16, tag="ob", name="ob")
                nc.vector.tensor_copy(ob[:], op[:])
                dma_eng = nc.scalar if nn == 1 else nc.sync
                dma_eng.dma_start(
                    out[ds(b * S + qi * 512 + c * 128, 128), ds(nn * 512, 512)],
                    ob[:])
            return run

        # ---- QKV ----
        def emit_transposes(vs, i, vaug):
            # 4 transposes into one psum tile, then 2 strided vaug copies
            vtp = qkv_ps.tile([128, 512], BF16, tag="qkv", name="vtp")
            for j4 in range(4):
                nc.tensor.transpose(vtp[:, ts(j4, 128)], vs[:, ts(j4, 128)],
                                    identr[:])

            def copies():
                v3d = vtp.rearrange("p (j c) -> p j c", j=4)
                nc.vector.tensor_copy(vaug[:, ds(4 * i, 4), 0:64],
                                      v3d[:, :, 0:64])
                nc.vector.tensor_copy(vaug[:, ds(4 * i, 4), 65:129],
                                      v3d[:, :, 64:128])
            return copies

        def qkv_batch(b, qt, kt, vaug):
            nc.vector.tensor_copy(vaug[:, :, 64:65].opt(), onesf[:, 0:NKC])
            nc.vector.tensor_copy(vaug[:, :, 129:130].opt(), onesf[:, 0:NKC])
            prev = None
            for i in range(NQ):
                emit_x_dma(b, i)
                xt = xt_tiles.pop((b, i))
                qp = qkv_ps.tile([128, 512], F32, tag="qkv", name="qp")
                for j in range(NDC):
                    nc.tensor.matmul(qp[:], wq_sb[:, j, :], xt[:, j, :],
                                     start=(j == 0), stop=(j == NDC - 1))
                pump_ctx(max(0, 4 - 2 * i))
                pump_aux(1)
                kp = qkv_ps.tile([128, 512], F32, tag="qkv", name="kp")
                for j in range(NDC):
                    nc.tensor.matmul(kp[:], wk_sb[:, j, :], xt[:, j, :],
                                     start=(j == 0), stop=(j == NDC - 1))
                nc.vector.tensor_copy(qt[:, ts(i, 512)], qp[:])
                nc.vector.tensor_copy(kt[:, ts(i, 512)], kp[:])
                pump_ctx(max(0, 3 - 2 * i))
                vcopies = None
                if prev is not None:
                    vcopies = emit_transposes(prev[0], prev[1], vaug)
                pump_aux(1)
                vp = qkv_ps.tile([128, 512], F32, tag="qkv", name="vp")
                for j in range(NDC):
                    nc.tensor.matmul(vp[:], wv_sb[:, j, :], xt[:, j, :],
                                     start=(j == 0), stop=(j == NDC - 1))
                vs = ep_sb.tile([128, 512], BF16, tag="vs", name="vs")
                nc.vector.tensor_copy(vs[:], vp[:])
                if vcopies is not None:
                    vcopies()
                prev = (vs, i)
                pump_ctx(max(0, 2 - 2 * i))
                pump_aux(1)
            return prev

        # ---- attention ----
        def attn_batch(b, qt, kt, vaug, ctxT):
            for qi in range(NQ):
                nk = 4 * qi + 4
                for hh in range(HL):
                    cp = cp_ps.tile([65, 512], F32, tag="cp", name="cp")
                    for p in range(nk // 2):
                        sp = sp_ps.tile([128, 1024], F32, tag="sp", name="sp")
                        et = et_sb.tile([128, 1024], BF16, tag="et", name="et")
                        kis = (2 * p, 2 * p + 1)
                        alos = []
                        for ki in kis:
                            a = ki - (nk - 4) if ki >= nk - 4 else -1
                            lo = 0 if a < 0 else a * 128
                            hb = 512 * (ki & 1)
                            alos.append((ki, a, lo, hb))
                            nc.tensor.matmul(
                                sp[:, ds(hb + lo, 512 - lo)],
                                kt[ds(64 * hh, 64), ts(ki, 128)].opt(),
                                qt[ds(64 * hh, 64),
                                   ds(qi * 512 + lo, 512 - lo)].opt(),
                                start=True, stop=True)
                        if alos[0][1] < 0 and alos[1][1] < 0:
                            nc.scalar.activation(et[:], sp[:], EXP, scale=0.125)
                        else:
                            for ki, a, lo, hb in alos:
                                nc.scalar.activation(
                                    et[:, ds(hb + lo, 512 - lo)],
                                    sp[:, ds(hb + lo, 512 - lo)],
                                    EXP, scale=0.125)
                                # zero upper triangle of the diagonal band:
                                # et[k, z] = 0 where z < k
                                nc.gpsimd.affine_select(
                                    out=et[:, ds(hb + lo, 128)],
                                    in_=et[:, ds(hb + lo, 128)],
                                    compare_op=mybir.AluOpType.is_ge,
                                    fill=0.0, base=0, pattern=[[1, 128]],
                                    channel_multiplier=-1)
                        for ki, a, lo, hb in alos:
                            def ctx_ki(ki=ki, a=a, lo=lo, hb=hb, et=et, cp=cp,
                                       hh=hh, vaug=vaug):
                                st = ki == 0
                                vsl = vaug[:, ki, ds(65 * hh, 65)]
                                if a < 0:
                                    nc.tensor.matmul(cp[:, :], vsl,
                                                     et[:, ds(hb, 512)],
                                                     start=st, stop=False)
                                else:
                                    nc.tensor.matmul(cp[:, ds(lo, 128)], vsl,
                                                     et[:, ds(hb + lo, 128)],
                                                     start=st, stop=True)
                                    if a < 3:
                                        nc.tensor.matmul(
                                            cp[:, ds(lo + 128, 384 - lo)], vsl,
                                            et[:, ds(hb + lo + 128, 384 - lo)],
                                            start=st, stop=False)
                            ctxq.append(ctx_ki)
                            pump_ctx(TRAIL)
                            tick()
                    # stage epilogue, deferred: A = 1/denom on DVE;
                    # B = broadcast matmul + normalize-mul + outproj enqueue
                    cell = {}

                    def partA(cp=cp, cell=cell):
                        # 1/denom = exp(-ln(denom)) on ACT
                        lg = ep_sb.tile([1, 512], F32, tag="lg", name="lg")
                        nc.scalar.activation(lg[:], cp[64:65, :], LN)
                        rr = ep_sb.tile([1, 512], BF16, tag="rr", name="rr")
                        nc.scalar.activation(rr[:], lg[:], EXP, scale=-1.0)
                        cell["rr"] = rr

                    def partB(qi=qi, hh=hh, cp=cp, cell=cell, ctxT=ctxT, b=b):
                        rr = cell["rr"]
                        bcp = qkv_ps.tile([64, 512], F32, tag="qkv", name="bcp")
                        nc.tensor.matmul(bcp[:], ones_lr[:], rr[:],
                                         start=True, stop=True)
                        bcs = ep_sb.tile([64, 512], F32, tag="bcs", name="bcs")
                        nc.vector.tensor_copy(bcs[:], bcp[:])
                        nc.vector.tensor_mul(
                            ctxT[ds(64 * hh, 64), ts(qi, 512)],
                            cp[0:64, :], bcs[:])
                        if hh == 1:
                            rdy = state["slot"] + 4
                            for c in range(4):
                                for nn in range(2):
                                    aux.append(
                                        (rdy, make_outproj(b, qi, c, nn, ctxT)))

                    ctxq.append(partA)
                    if state["partB"] is not None:
                        ctxq.append(state["partB"])
                    state["partB"] = partB
                if qi == 1:
                    emit_x_dma(b + 1, 1)

        # ---- main ----
        emit_x_dma(0, 0, split=True)
        nc.sync.dma_start(wk_sb[:], wk.rearrange("(j p) h -> p j h", p=128))
        nc.sync.dma_start(wv_sb[:], wv.rearrange("(j p) h -> p j h", p=128))
        emit_x_dma(0, 1)
        nc.sync.dma_start(wo_sb[:], wo[:, :])
        for b in range(B):
            qt = qk_sb.tile([128, S], BF16, tag="qt", name="qt")
            kt = qk_sb.tile([128, S], BF16, tag="kt", name="kt")
            vaug = vpool.tile([128, NKC, 130], BF16, name="vaug")
            ctxT = ctx_sb.tile([128, S], BF16, tag="ctx", name="ctxT")
            prev = qkv_batch(b, qt, kt, vaug)
            emit_x_dma(b + 1, 0)
            emit_transposes(prev[0], prev[1], vaug)()
            attn_batch(b, qt, kt, vaug, ctxT)
        # flush
        pump_ctx(0)
        if state["partB"] is not None:
            state["partB"]()
            state["partB"] = None
        pump_ctx(0)
        while aux:
            pump_aux(1, force=True)

    return nc


# ======== host-side wrapper ========
_CACHE = {}


def _get_program():
    if "nc" not in _CACHE:
        install()
        _CACHE["nc"] = build()
    return _CACHE["nc"]


def _run(inputs, trace=False):
    import ml_dtypes
    from concourse.bass_utils import run_bass_kernel_spmd

    bf16 = ml_dtypes.bfloat16
    x = np.asarray(inputs["x"], dtype=np.float32)
    WQ = np.asarray(inputs["WQ"], dtype=np.float32)
    WK = np.asarray(inputs["WK"], dtype=np.float32)
    WV = np.asarray(inputs["WV"], dtype=np.float32)
    WO = np.asarray(inputs["WO"], dtype=np.float32)

    xTh = np.ascontiguousarray(x.reshape(BS, D).T.astype(bf16))
    woT = WO.T.astype(bf16)
    in_maps = []
    for c in range(NC):
        sl = slice(c * 128, (c + 1) * 128)
        in_maps.append({
            "xT": xTh,
            "wq": np.ascontiguousarray(WQ[sl, :].T.astype(bf16)),
            "wk": np.ascontiguousarray(WK[sl, :].T.astype(bf16)),
            "wv": np.ascontiguousarray(WV[sl, :].T.astype(bf16)),
            "wo": np.ascontiguousarray(woT[sl, :]),
        })

    nc_prog = _get_program()
    res = run_bass_kernel_spmd(nc_prog, in_maps, list(range(NC)), trace=trace)

    actual = np.zeros((BS, D), dtype=np.float32)
    for c in range(NC):
        actual += np.asarray(res.results[c]["out"], dtype=np.float32)
    return actual.reshape(x.shape), res


def kernel(**inputs):
    out, _ = _run(inputs, trace=False)
    return out


# revision 48
# speedup vs baseline: 1.0014x; 1.0014x over previous
"""Self-contained Trainium2 kernel for nn_MultiHeadAttention_91070486544496.

B=4, S=2048, D=1024, H=16 causal MHA. 8-core SPMD, head-parallel:
each core computes QKV+attention for its 2 heads over all batches, then
multiplies its context slice by the matching WO rows and writes a full
[BS, D] partial output; the host sums the 8 partials. No collectives.

Schedule keeps the TensorEngine gap-free (DVFS ramps 1.2->2.4GHz after
3us of continuous busy): ctx matmuls trail the score stream via a
deferred-work queue, V-transposes are deferred one chunk, softmax
normalization (reciprocal on DVE, broadcast matmul, multiply on Pool)
trails by a full head-stage, and output-projection matmuls are injected
as PE filler between score pairs. Causal masking is done by trimming
fully-masked column bands (never computed) and zeroing the partially
masked 128-wide diagonal band of exp(scores) with a Pool affine_select.
"""
import sys

for _p in ("/opt/trn_rl_repo", "/root/.axon_site/_ro/trn_rl_repo"):
    if _p not in sys.path:
        sys.path.append(_p)

import numpy as np

# ======== runtime infra (axon NTFF hook, BIR wait splitter) ========

import contextlib
import ctypes
import json
import types

_SO_PATH = "/opt/axon/libaxon_pjrt.so"


def _ntff_profile_via_ctypes(so_path):
    lib = ctypes.CDLL(so_path)
    if not hasattr(lib, "axon_start_nrt_profile"):
        return None
    lib.axon_start_nrt_profile.argtypes = [
        ctypes.POINTER(ctypes.c_int64),
        ctypes.c_size_t,
    ]
    lib.axon_start_nrt_profile.restype = ctypes.c_int64
    lib.axon_stop_nrt_profile.argtypes = [ctypes.c_char_p]
    lib.axon_stop_nrt_profile.restype = ctypes.c_int64

    @contextlib.contextmanager
    def _hook(output_dir, device_ids):
        import jax
        jax.devices()
        if device_ids:
            ids = (ctypes.c_int64 * len(device_ids))(*device_ids)
            rc = lib.axon_start_nrt_profile(ids, len(device_ids))
        else:
            rc = lib.axon_start_nrt_profile(None, 0)
        if rc != 0:
            raise RuntimeError(f"axon_start_nrt_profile rc={rc}")
        try:
            yield
        finally:
            n = lib.axon_stop_nrt_profile(str(output_dir).encode())
            if n < 0:
                raise RuntimeError(f"axon_stop_nrt_profile rc={n}")

    return _hook


def split_multi_waits(bir_json: bytes) -> bytes:
    d = json.loads(bir_json)
    n_split = 0
    for fn in d.get("functions", []):
        for blk in fn.get("blocks", []):
            insts = blk.get("instructions", [])
            out = []
            for inst in insts:
                si = inst.get("sync_info")
                waits = (si or {}).get("on_wait") or []
                if len(waits) > 1:
                    extra, keep = waits[:-1], waits[-1:]
                    for k, w in enumerate(extra):
                        out.append({
                            "debug": inst.get("debug", 0),
                            "engine": inst["engine"],
                            "ins": [],
                            "outs": [],
                            "name": f"{inst['name']}-ws{k}",
                            "opcode": "NoOp",
                            "sync_info": {"on_update": [], "on_wait": [w]},
                        })
                        n_split += 1
                    si["on_wait"] = keep
                out.append(inst)
            blk["instructions"] = out
    if n_split:
        print(f"bass_infra: split {n_split} extra sync waits into NoOps")
    return json.dumps(d).encode()


def install():
    # 1. antenv.axon_hooks shim
    if "antenv.axon_hooks" not in sys.modules:
        mod = types.ModuleType("antenv.axon_hooks")
        _state = {"hook": _ntff_profile_via_ctypes(_SO_PATH)}
        mod.set_axon_ntff_profile_hook = lambda h: _state.__setitem__("hook", h)
        mod.get_axon_ntff_profile_hook = lambda: _state["hook"]
        sys.modules["antenv.axon_hooks"] = mod
        import antenv
        antenv.axon_hooks = mod

    from concourse import bass_utils, bass2jax

    # 2. upload_artifacts stub
    bass_utils.upload_artifacts = lambda tmpdir: tmpdir

    # 3. wait-splitting compile wrapper
    orig_compile = bass_utils.compile_bir_kernel

    def compile_with_split(bir_json, tmpdir, neff_name="file.neff"):
        return orig_compile(split_multi_waits(bir_json), tmpdir, neff_name=neff_name)

    if getattr(bass2jax.compile_bir_kernel, "__name__", "") != "compile_with_split":
        bass_utils.compile_bir_kernel = compile_with_split
        bass2jax.compile_bir_kernel = compile_with_split


# ======== kernel IR builder ========
from collections import deque
from contextlib import ExitStack

import concourse.bass as bass
import concourse.mybir as mybir
import concourse.tile as tile
from concourse.bass import ds, ts
from concourse.masks import make_identity

F32 = mybir.dt.float32
BF16 = mybir.dt.bfloat16
EXP = mybir.ActivationFunctionType.Exp
LN = mybir.ActivationFunctionType.Ln

B, S, D, H, DK = 4, 2048, 1024, 16, 64
NC = 8          # cores
HL = 2          # heads per core
BS = B * S      # 8192
NQ = S // 512   # q-chunks per batch = 4
NKC = S // 128  # k-chunks per batch = 16
NDC = D // 128  # d_in chunks = 8
TRAIL = 7       # deferred-work queue depth (ki items)


def build(cfg=None):
    nc = bass.Bass("TRN2", target_bir_lowering=False, debug=False, num_devices=NC)

    xT = nc.dram_tensor("xT", [D, BS], BF16, kind="ExternalInput")
    wq = nc.dram_tensor("wq", [D, 128], BF16, kind="ExternalInput")
    wk = nc.dram_tensor("wk", [D, 128], BF16, kind="ExternalInput")
    wv = nc.dram_tensor("wv", [D, 128], BF16, kind="ExternalInput")
    wo = nc.dram_tensor("wo", [128, D], BF16, kind="ExternalInput")
    out = nc.dram_tensor("out", [BS, D], BF16, kind="ExternalOutput")

    with tile.TileContext(nc) as tc, ExitStack() as ctx:
        const = ctx.enter_context(tc.tile_pool(name="const", bufs=1))
        wpool = ctx.enter_context(tc.tile_pool(name="wpool", bufs=1))
        xpool = ctx.enter_context(tc.tile_pool(name="xpool", bufs=3))
        qkv_ps = ctx.enter_context(tc.tile_pool(name="qkv_ps", bufs=2, space="PSUM"))
        sp_ps = ctx.enter_context(tc.tile_pool(name="sp_ps", bufs=2, space="PSUM"))
        cp_ps = ctx.enter_context(tc.tile_pool(name="cp_ps", bufs=2, space="PSUM"))
        qk_sb = ctx.enter_context(tc.tile_pool(name="qk_sb", bufs=1))
        vpool = ctx.enter_context(tc.tile_pool(name="vpool", bufs=2))
        et_sb = ctx.enter_context(tc.tile_pool(name="et_sb", bufs=12))
        ep_sb = ctx.enter_context(tc.tile_pool(name="ep_sb", bufs=2))
        ctx_sb = ctx.enter_context(tc.tile_pool(name="ctx_sb", bufs=2))
        ob_sb = ctx.enter_context(tc.tile_pool(name="ob_sb", bufs=4))

        # ---- constants ----
        ident = const.tile([128, 128], F32)
        make_identity(nc, ident[:])
        identr = const.tile([128, 128], BF16)
        nc.vector.tensor_copy(identr[:], ident[:])
        onesf = const.tile([128, 16], F32)
        nc.vector.memset(onesf[:], 1.0)
        ones_l = const.tile([1, 64], F32)
        nc.vector.memset(ones_l[:], 1.0)
        ones_lr = const.tile([1, 64], BF16)
        nc.vector.tensor_copy(ones_lr[:], ones_l[:])
        ones_row = const.tile([1, 512], F32)
        nc.vector.memset(ones_row[:], 1.0)

        # ---- weights ----
        wq_sb = wpool.tile([128, NDC, 128], BF16)
        wk_sb = wpool.tile([128, NDC, 128], BF16)
        wv_sb = wpool.tile([128, NDC, 128], BF16)
        wo_sb = wpool.tile([128, D], BF16)
        nc.sync.dma_start(wq_sb[:], wq.rearrange("(j p) h -> p j h", p=128))

        # ---- deferred work ----
        ctxq = deque()   # trailing ctx matmuls + softmax epilogue parts
        aux = deque()    # out-projection bundles (PE filler)
        state = {"partB": None, "slot": 0}

        def pump_ctx(target):
            while len(ctxq) > target:
                ctxq.popleft()()

        def pump_aux(n=1, force=False):
            for _ in range(n):
                if not aux:
                    return
                if not force and aux[0][0] > state["slot"]:
                    return
                aux.popleft()[1]()

        def tick():
            state["slot"] += 1
            if state["slot"] % 2 == 0:
                pump_aux(1)

        # ---- x prefetch ----
        xt_tiles = {}

        def emit_x_dma(b, i, split=False):
            if b >= B or (b, i) in xt_tiles:
                return
            t = xpool.tile([128, NDC, 512], BF16, tag="xt", name=f"xt{b}_{i}")
            xt_tiles[(b, i)] = t
            src = xT.rearrange("(j p) n -> p j n", p=128)[:, :, ds(b * S + i * 512, 512)]
            eng = nc.scalar if b == 0 and i < 2 else nc.sync
            if split:
                for j in range(NDC):
                    eng.dma_start(t[:, j, :], src[:, j, :])
            else:
                eng.dma_start(t[:], src)

        # ---- out-projection bundle: one PE matmul + Pool copy + DMA ----
        def make_outproj(b, qi, c, nn, ctxT):
            def run():
                op = qkv_ps.tile([128, 512], F32, tag="qkv", name="op")
                nc.tensor.matmul(op[:], ctxT[:, ds(qi * 512 + c * 128, 128)],
                                 wo_sb[:, ds(nn * 512, 512)], start=True, stop=True)
                ob = ob_sb.tile([128, 512], BF16, tag="ob", name="ob")
                nc.vector.tensor_copy(ob[:], op[:])
                eng = nc.scalar if (b == B - 1 and qi == 3 and nn == 1) else nc.sync
                eng.dma_start(
                    out[ds(b * S + qi * 512 + c * 128, 128), ds(nn * 512, 512)],
                    ob[:])
            return run

        # ---- QKV ----
        def emit_transposes(vs, i, vaug):
            # 4 transposes into one psum tile, then 2 strided vaug copies
            vtp = qkv_ps.tile([128, 512], BF16, tag="qkv", name="vtp")
            for j4 in range(4):
                nc.tensor.transpose(vtp[:, ts(j4, 128)], vs[:, ts(j4, 128)],
                                    identr[:])

            def copies():
                v3d = vtp.rearrange("p (j c) -> p j c", j=4)
                nc.vector.tensor_copy(vaug[:, ds(4 * i, 4), 0:64],
                                      v3d[:, :, 0:64])
                nc.vector.tensor_copy(vaug[:, ds(4 * i, 4), 65:129],
                                      v3d[:, :, 64:128])
            return copies

        def qkv_batch(b, qt, kt, vaug):
            nc.vector.tensor_copy(vaug[:, :, 64:65].opt(), onesf[:, 0:NKC])
            nc.vector.tensor_copy(vaug[:, :, 129:130].opt(), onesf[:, 0:NKC])
            prev = None
            for i in range(NQ):
                emit_x_dma(b, i)
                xt = xt_tiles.pop((b, i))
                qp = qkv_ps.tile([128, 512], F32, tag="qkv", name="qp")
                for j in range(NDC):
                    nc.tensor.matmul(qp[:], wq_sb[:, j, :], xt[:, j, :],
                                     start=(j == 0), stop=(j == NDC - 1))
                pump_ctx(max(0, 4 - 2 * i))
                pump_aux(1)
                kp = qkv_ps.tile([128, 512], F32, tag="qkv", name="kp")
                for j in range(NDC):
                    nc.tensor.matmul(kp[:], wk_sb[:, j, :], xt[:, j, :],
                                     start=(j == 0), stop=(j == NDC - 1))
                nc.vector.tensor_copy(qt[:, ts(i, 512)], qp[:])
                nc.vector.tensor_copy(kt[:, ts(i, 512)], kp[:])
                pump_ctx(max(0, 3 - 2 * i))
                vcopies = None
                if prev is not None:
                    vcopies = emit_transposes(prev[0], prev[1], vaug)
                pump_aux(1)
                vp = qkv_ps.tile([128, 512], F32, tag="qkv", name="vp")
                for j in range(NDC):
                    nc.tensor.matmul(vp[:], wv_sb[:, j, :], xt[:, j, :],
                                     start=(j == 0), stop=(j == NDC - 1))
                vs = ep_sb.tile([128, 512], BF16, tag="vs", name="vs")
                nc.vector.tensor_copy(vs[:], vp[:])
                if vcopies is not None:
                    vcopies()
                prev = (vs, i)
                pump_ctx(max(0, 2 - 2 * i))
                pump_aux(1)
            return prev

        # ---- attention ----
        def attn_batch(b, qt, kt, vaug, ctxT):
            for qi in range(NQ):
                nk = 4 * qi + 4
                for hh in range(HL):
                    cp = cp_ps.tile([65, 512], F32, tag="cp", name="cp")
                    for p in range(nk // 2):
                        sp = sp_ps.tile([128, 1024], F32, tag="sp", name="sp")
                        et = et_sb.tile([128, 1024], BF16, tag="et", name="et")
                        kis = (2 * p, 2 * p + 1)
                        alos = []
                        for ki in kis:
                            a = ki - (nk - 4) if ki >= nk - 4 else -1
                            lo = 0 if a < 0 else a * 128
                            hb = 512 * (ki & 1)
                            alos.append((ki, a, lo, hb))
                            nc.tensor.matmul(
                                sp[:, ds(hb + lo, 512 - lo)],
                                kt[ds(64 * hh, 64), ts(ki, 128)].opt(),
                                qt[ds(64 * hh, 64),
                                   ds(qi * 512 + lo, 512 - lo)].opt(),
                                start=True, stop=True)
                        if alos[0][1] < 0 and alos[1][1] < 0:
                            nc.scalar.activation(et[:], sp[:], EXP, scale=0.125)
                        else:
                            for ki, a, lo, hb in alos:
                                nc.scalar.activation(
                                    et[:, ds(hb + lo, 512 - lo)],
                                    sp[:, ds(hb + lo, 512 - lo)],
                                    EXP, scale=0.125)
                                # zero upper triangle of the diagonal band:
                                # et[k, z] = 0 where z < k
                                nc.gpsimd.affine_select(
                                    out=et[:, ds(hb + lo, 128)],
                                    in_=et[:, ds(hb + lo, 128)],
                                    compare_op=mybir.AluOpType.is_ge,
                                    fill=0.0, base=0, pattern=[[1, 128]],
                                    channel_multiplier=-1)
                        for ki, a, lo, hb in alos:
                            def ctx_ki(ki=ki, a=a, lo=lo, hb=hb, et=et, cp=cp,
                                       hh=hh, vaug=vaug):
                                st = ki == 0
                                vsl = vaug[:, ki, ds(65 * hh, 65)]
                                if a < 0:
                                    nc.tensor.matmul(cp[:, :], vsl,
                                                     et[:, ds(hb, 512)],
                                                     start=st, stop=False)
                                else:
                                    nc.tensor.matmul(cp[:, ds(lo, 128)], vsl,
                                                     et[:, ds(hb + lo, 128)],
                                                     start=st, stop=True)
                                    if a < 3:
                                        nc.tensor.matmul(
                                            cp[:, ds(lo + 128, 384 - lo)], vsl,
                                            et[:, ds(hb + lo + 128, 384 - lo)],
                                            start=st, stop=False)
                            ctxq.append(ctx_ki)
                            pump_ctx(TRAIL)
                            tick()
                    # stage epilogue, deferred: A = 1/denom on DVE;
                    # B = broadcast matmul + normalize-mul + outproj enqueue
                    cell = {}

                    def partA(cp=cp, cell=cell):
                        # 1/denom = exp(-ln(denom)) on ACT
                        lg = ep_sb.tile([1, 512], F32, tag="lg", name="lg")
                        nc.scalar.activation(lg[:], cp[64:65, :], LN)
                        rr = ep_sb.tile([1, 512], BF16, tag="rr", name="rr")
                        nc.scalar.activation(rr[:], lg[:], EXP, scale=-1.0)
                        cell["rr"] = rr

                    def partB(qi=qi, hh=hh, cp=cp, cell=cell, ctxT=ctxT, b=b):
                        rr = cell["rr"]
                        bcp = qkv_ps.tile([64, 512], F32, tag="qkv", name="bcp")
                        nc.tensor.matmul(bcp[:], ones_lr[:], rr[:],
                                         start=True, stop=True)
                        bcs = ep_sb.tile([64, 512], F32, tag="bcs", name="bcs")
                        nc.vector.tensor_copy(bcs[:], bcp[:])
                        nc.vector.tensor_mul(
                            ctxT[ds(64 * hh, 64), ts(qi, 512)],
                            cp[0:64, :], bcs[:])
                        if hh == 1:
                            rdy = state["slot"] + 4
                            for c in range(4):
                                for nn in range(2):
                                    aux.append(
                                        (rdy, make_outproj(b, qi, c, nn, ctxT)))

                    ctxq.append(partA)
                    if state["partB"] is not None:
                        ctxq.append(state["partB"])
                    state["partB"] = partB
                if qi == 1:
                    emit_x_dma(b + 1, 1)

        # ---- main ----
        emit_x_dma(0, 0, split=True)
        nc.sync.dma_start(wk_sb[:], wk.rearrange("(j p) h -> p j h", p=128))
        nc.sync.dma_start(wv_sb[:], wv.rearrange("(j p) h -> p j h", p=128))
        emit_x_dma(0, 1)
        nc.sync.dma_start(wo_sb[:], wo[:, :])
        for b in range(B):
            qt = qk_sb.tile([128, S], BF16, tag="qt", name="qt")
            kt = qk_sb.tile([128, S], BF16, tag="kt", name="kt")
            vaug = vpool.tile([128, NKC, 130], BF16, name="vaug")
            ctxT = ctx_sb.tile([128, S], BF16, tag="ctx", name="ctxT")
            prev = qkv_batch(b, qt, kt, vaug)
            emit_x_dma(b + 1, 0)
            emit_transposes(prev[0], prev[1], vaug)()
            attn_batch(b, qt, kt, vaug, ctxT)
        # flush
        pump_ctx(0)
        if state["partB"] is not None:
            state["partB"]()
            state["partB"] = None
        pump_ctx(0)
        while aux:
            pump_aux(1, force=True)

    return nc


# ======== host-side wrapper ========
_CACHE = {}


def _get_program():
    if "nc" not in _CACHE:
        install()
        _CACHE["nc"] = build()
    return _CACHE["nc"]


def _run(inputs, trace=False):
    import ml_dtypes
    from concourse.bass_utils import run_bass_kernel_spmd

    bf16 = ml_dtypes.bfloat16
    x = np.asarray(inputs["x"], dtype=np.float32)
    WQ = np.asarray(inputs["WQ"], dtype=np.float32)
    WK = np.asarray(inputs["WK"], dtype=np.float32)
    WV = np.asarray(inputs["WV"], dtype=np.float32)
    WO = np.asarray(inputs["WO"], dtype=np.float32)

    xTh = np.ascontiguousarray(x.reshape(BS, D).T.astype(bf16))
    woT = WO.T.astype(bf16)
    in_maps = []
    for c in range(NC):
        sl = slice(c * 128, (c + 1) * 128)
        in_maps.append({
            "xT": xTh,
            "wq": np.ascontiguousarray(WQ[sl, :].T.astype(bf16)),
            "wk": np.ascontiguousarray(WK[sl, :].T.astype(bf16)),
            "wv": np.ascontiguousarray(WV[sl, :].T.astype(bf16)),
            "wo": np.ascontiguousarray(woT[sl, :]),
        })

    nc_prog = _get_program()
    res = run_bass_kernel_spmd(nc_prog, in_maps, list(range(NC)), trace=trace)

    actual = np.zeros((BS, D), dtype=np.float32)
    for c in range(NC):
        actual += np.asarray(res.results[c]["out"], dtype=np.float32)
    return actual.reshape(x.shape), res


def kernel(**inputs):
    out, _ = _run(inputs, trace=False)
    return out


# revision 51
# speedup vs baseline: 1.0399x; 1.0385x over previous
"""Self-contained Trainium2 kernel for nn_MultiHeadAttention_91070486544496.

B=4, S=2048, D=1024, H=16 causal MHA. 8-core SPMD, head-parallel:
each core computes QKV+attention for its 2 heads over all batches, then
multiplies its context slice by the matching WO rows and writes a full
[BS, D] partial output; the host sums the 8 partials. No collectives.

Schedule keeps the TensorEngine gap-free (DVFS ramps 1.2->2.4GHz after
3us of continuous busy): ctx matmuls trail the score stream via a
deferred-work queue, V-transposes are deferred one chunk, softmax
normalization (reciprocal on DVE, broadcast matmul, multiply on Pool)
trails by a full head-stage, and output-projection matmuls are injected
as PE filler between score pairs. Causal masking is done by trimming
fully-masked column bands (never computed) and zeroing the partially
masked 128-wide diagonal band of exp(scores) with a Pool affine_select.
"""
import sys

for _p in ("/opt/trn_rl_repo", "/root/.axon_site/_ro/trn_rl_repo"):
    if _p not in sys.path:
        sys.path.append(_p)

import numpy as np

# ======== runtime infra (axon NTFF hook, BIR wait splitter) ========

import contextlib
import ctypes
import json
import types

_SO_PATH = "/opt/axon/libaxon_pjrt.so"


def _ntff_profile_via_ctypes(so_path):
    lib = ctypes.CDLL(so_path)
    if not hasattr(lib, "axon_start_nrt_profile"):
        return None
    lib.axon_start_nrt_profile.argtypes = [
        ctypes.POINTER(ctypes.c_int64),
        ctypes.c_size_t,
    ]
    lib.axon_start_nrt_profile.restype = ctypes.c_int64
    lib.axon_stop_nrt_profile.argtypes = [ctypes.c_char_p]
    lib.axon_stop_nrt_profile.restype = ctypes.c_int64

    @contextlib.contextmanager
    def _hook(output_dir, device_ids):
        import jax
        jax.devices()
        if device_ids:
            ids = (ctypes.c_int64 * len(device_ids))(*device_ids)
            rc = lib.axon_start_nrt_profile(ids, len(device_ids))
        else:
            rc = lib.axon_start_nrt_profile(None, 0)
        if rc != 0:
            raise RuntimeError(f"axon_start_nrt_profile rc={rc}")
        try:
            yield
        finally:
            n = lib.axon_stop_nrt_profile(str(output_dir).encode())
            if n < 0:
                raise RuntimeError(f"axon_stop_nrt_profile rc={n}")

    return _hook


def split_multi_waits(bir_json: bytes) -> bytes:
    d = json.loads(bir_json)
    n_split = 0
    for fn in d.get("functions", []):
        for blk in fn.get("blocks", []):
            insts = blk.get("instructions", [])
            out = []
            for inst in insts:
                si = inst.get("sync_info")
                waits = (si or {}).get("on_wait") or []
                if len(waits) > 1:
                    extra, keep = waits[:-1], waits[-1:]
                    for k, w in enumerate(extra):
                        out.append({
                            "debug": inst.get("debug", 0),
                            "engine": inst["engine"],
                            "ins": [],
                            "outs": [],
                            "name": f"{inst['name']}-ws{k}",
                            "opcode": "NoOp",
                            "sync_info": {"on_update": [], "on_wait": [w]},
                        })
                        n_split += 1
                    si["on_wait"] = keep
                out.append(inst)
            blk["instructions"] = out
    if n_split:
        print(f"bass_infra: split {n_split} extra sync waits into NoOps")
    return json.dumps(d).encode()


def install():
    # 1. antenv.axon_hooks shim
    if "antenv.axon_hooks" not in sys.modules:
        mod = types.ModuleType("antenv.axon_hooks")
        _state = {"hook": _ntff_profile_via_ctypes(_SO_PATH)}
        mod.set_axon_ntff_profile_hook = lambda h: _state.__setitem__("hook", h)
        mod.get_axon_ntff_profile_hook = lambda: _state["hook"]
        sys.modules["antenv.axon_hooks"] = mod
        import antenv
        antenv.axon_hooks = mod

    from concourse import bass_utils, bass2jax

    # 2. upload_artifacts stub
    bass_utils.upload_artifacts = lambda tmpdir: tmpdir

    # 3. wait-splitting compile wrapper
    orig_compile = bass_utils.compile_bir_kernel

    def compile_with_split(bir_json, tmpdir, neff_name="file.neff"):
        return orig_compile(split_multi_waits(bir_json), tmpdir, neff_name=neff_name)

    if getattr(bass2jax.compile_bir_kernel, "__name__", "") != "compile_with_split":
        bass_utils.compile_bir_kernel = compile_with_split
        bass2jax.compile_bir_kernel = compile_with_split


# ======== kernel IR builder ========
from collections import deque
from contextlib import ExitStack

import concourse.bass as bass
import concourse.mybir as mybir
import concourse.tile as tile
from concourse.bass import ds, ts
from concourse.masks import make_identity

F32 = mybir.dt.float32
BF16 = mybir.dt.bfloat16
EXP = mybir.ActivationFunctionType.Exp
LN = mybir.ActivationFunctionType.Ln

B, S, D, H, DK = 4, 2048, 1024, 16, 64
NC = 8          # cores
HL = 2          # heads per core
BS = B * S      # 8192
NQ = S // 512   # q-chunks per batch = 4
NKC = S // 128  # k-chunks per batch = 16
NDC = D // 128  # d_in chunks = 8
TRAIL = 7       # deferred-work queue depth (ki items)


def build(cfg=None):
    nc = bass.Bass("TRN2", target_bir_lowering=False, debug=False, num_devices=NC)

    xT = nc.dram_tensor("xT", [D, BS], BF16, kind="ExternalInput")
    wq = nc.dram_tensor("wq", [D, 128], BF16, kind="ExternalInput")
    wk = nc.dram_tensor("wk", [D, 128], BF16, kind="ExternalInput")
    wv = nc.dram_tensor("wv", [D, 128], BF16, kind="ExternalInput")
    wo = nc.dram_tensor("wo", [128, D], BF16, kind="ExternalInput")
    out = nc.dram_tensor("out", [BS, D], BF16, kind="ExternalOutput")

    with tile.TileContext(nc) as tc, ExitStack() as ctx:
        const = ctx.enter_context(tc.tile_pool(name="const", bufs=1))
        wpool = ctx.enter_context(tc.tile_pool(name="wpool", bufs=1))
        xpool = ctx.enter_context(tc.tile_pool(name="xpool", bufs=3))
        qkv_ps = ctx.enter_context(tc.tile_pool(name="qkv_ps", bufs=2, space="PSUM"))
        sp_ps = ctx.enter_context(tc.tile_pool(name="sp_ps", bufs=2, space="PSUM"))
        cp_ps = ctx.enter_context(tc.tile_pool(name="cp_ps", bufs=2, space="PSUM"))
        qk_sb = ctx.enter_context(tc.tile_pool(name="qk_sb", bufs=1))
        vpool = ctx.enter_context(tc.tile_pool(name="vpool", bufs=2))
        et_sb = ctx.enter_context(tc.tile_pool(name="et_sb", bufs=12))
        ep_sb = ctx.enter_context(tc.tile_pool(name="ep_sb", bufs=2))
        ctx_sb = ctx.enter_context(tc.tile_pool(name="ctx_sb", bufs=2))
        ob_sb = ctx.enter_context(tc.tile_pool(name="ob_sb", bufs=4))

        # ---- constants ----
        ident = const.tile([128, 128], F32)
        make_identity(nc, ident[:])
        identr = const.tile([128, 128], BF16)
        nc.vector.tensor_copy(identr[:], ident[:])
        onesf = const.tile([128, 16], F32)
        nc.vector.memset(onesf[:], 1.0)
        ones_l = const.tile([1, 64], F32)
        nc.vector.memset(ones_l[:], 1.0)
        ones_lr = const.tile([1, 64], BF16)
        nc.vector.tensor_copy(ones_lr[:], ones_l[:])
        ones_row = const.tile([1, 512], F32)
        nc.vector.memset(ones_row[:], 1.0)

        # ---- weights ----
        wq_sb = wpool.tile([128, NDC, 128], BF16)
        wk_sb = wpool.tile([128, NDC, 128], BF16)
        wv_sb = wpool.tile([128, NDC, 128], BF16)
        wo_sb = wpool.tile([128, D], BF16)
        nc.sync.dma_start(wq_sb[:], wq.rearrange("(j p) h -> p j h", p=128))

        # ---- deferred work ----
        ctxq = deque()   # trailing ctx matmuls + softmax epilogue parts
        aux = deque()    # out-projection bundles (PE filler)
        state = {"partB": None, "slot": 0}

        def pump_ctx(target):
            while len(ctxq) > target:
                ctxq.popleft()()

        def pump_aux(n=1, force=False):
            for _ in range(n):
                if not aux:
                    return
                if not force and aux[0][0] > state["slot"]:
                    return
                aux.popleft()[1]()

        def tick():
            state["slot"] += 1
            if state["slot"] % 2 == 0:
                pump_aux(1)

        # ---- x prefetch ----
        xt_tiles = {}

        def emit_x_dma(b, i, split=False):
            if b >= B or (b, i) in xt_tiles:
                return
            t = xpool.tile([128, NDC, 512], BF16, tag="xt", name=f"xt{b}_{i}")
            xt_tiles[(b, i)] = t
            src = xT.rearrange("(j p) n -> p j n", p=128)[:, :, ds(b * S + i * 512, 512)]
            if split:
                for j in range(NDC):
                    nc.sync.dma_start(t[:, j, :], src[:, j, :])
            else:
                nc.sync.dma_start(t[:], src)

        # ---- out-projection bundle: one PE matmul + Pool copy + DMA ----
        def make_outproj(b, qi, c, nn, ctxT):
            def run():
                op = qkv_ps.tile([128, 512], F32, tag="qkv", name="op")
                nc.tensor.matmul(op[:], ctxT[:, ds(qi * 512 + c * 128, 128)],
                                 wo_sb[:, ds(nn * 512, 512)], start=True, stop=True)
                ob = ob_sb.tile([128, 512], BF16, tag="ob", name="ob")
                nc.vector.tensor_copy(ob[:], op[:])
                nc.sync.dma_start(
                    out[ds(b * S + qi * 512 + c * 128, 128), ds(nn * 512, 512)],
                    ob[:])
            return run

        # ---- QKV ----
        def emit_transposes(vs, i, vaug):
            # 4 transposes into one psum tile, then 2 strided vaug copies
            vtp = qkv_ps.tile([128, 512], BF16, tag="qkv", name="vtp")
            for j4 in range(4):
                nc.tensor.transpose(vtp[:, ts(j4, 128)], vs[:, ts(j4, 128)],
                                    identr[:])

            def copies():
                v3d = vtp.rearrange("p (j c) -> p j c", j=4)
                nc.vector.tensor_copy(vaug[:, ds(4 * i, 4), 0:64],
                                      v3d[:, :, 0:64])
                nc.vector.tensor_copy(vaug[:, ds(4 * i, 4), 65:129],
                                      v3d[:, :, 64:128])
            return copies

        def qkv_batch(b, qt, kt, vaug):
            nc.vector.tensor_copy(vaug[:, :, 64:65].opt(), onesf[:, 0:NKC])
            nc.vector.tensor_copy(vaug[:, :, 129:130].opt(), onesf[:, 0:NKC])
            prev = None
            for i in range(NQ):
                emit_x_dma(b, i)
                xt = xt_tiles.pop((b, i))
                qp = qkv_ps.tile([128, 512], F32, tag="qkv", name="qp")
                for j in range(NDC):
                    nc.tensor.matmul(qp[:], wq_sb[:, j, :], xt[:, j, :],
                                     start=(j == 0), stop=(j == NDC - 1))
                pump_ctx(max(0, 4 - 2 * i))
                pump_aux(1)
                kp = qkv_ps.tile([128, 512], F32, tag="qkv", name="kp")
                for j in range(NDC):
                    nc.tensor.matmul(kp[:], wk_sb[:, j, :], xt[:, j, :],
                                     start=(j == 0), stop=(j == NDC - 1))
                nc.vector.tensor_copy(qt[:, ts(i, 512)], qp[:])
                nc.vector.tensor_copy(kt[:, ts(i, 512)], kp[:])
                pump_ctx(max(0, 3 - 2 * i))
                vcopies = None
                if prev is not None:
                    vcopies = emit_transposes(prev[0], prev[1], vaug)
                pump_aux(1)
                vp = qkv_ps.tile([128, 512], F32, tag="qkv", name="vp")
                for j in range(NDC):
                    nc.tensor.matmul(vp[:], wv_sb[:, j, :], xt[:, j, :],
                                     start=(j == 0), stop=(j == NDC - 1))
                vs = ep_sb.tile([128, 512], BF16, tag="vs", name="vs")
                nc.vector.tensor_copy(vs[:], vp[:])
                if vcopies is not None:
                    vcopies()
                prev = (vs, i)
                pump_ctx(max(0, 2 - 2 * i))
                pump_aux(1)
            return prev

        # ---- attention ----
        def attn_batch(b, qt, kt, vaug, ctxT):
            for qi in range(NQ):
                nk = 4 * qi + 4
                for hh in range(HL):
                    cp = cp_ps.tile([65, 512], F32, tag="cp", name="cp")
                    for p in range(nk // 2):
                        sp = sp_ps.tile([128, 1024], F32, tag="sp", name="sp")
                        et = et_sb.tile([128, 1024], BF16, tag="et", name="et")
                        k0, k1 = 2 * p, 2 * p + 1
                        if k0 < nk - 4:
                            # non-diagonal pair: full-width halves, one exp
                            for ki in (k0, k1):
                                hb = 512 * (ki & 1)
                                nc.tensor.matmul(
                                    sp[:, ds(hb, 512)],
                                    kt[ds(64 * hh, 64), ts(ki, 128)].opt(),
                                    qt[ds(64 * hh, 64), ts(qi, 512)].opt(),
                                    start=True, stop=True)
                            nc.scalar.activation(et[:], sp[:], EXP, scale=0.125)
                            for ki in (k0, k1):
                                hb = 512 * (ki & 1)

                                def ctx_ki(ki=ki, hb=hb, et=et, cp=cp, hh=hh,
                                           vaug=vaug):
                                    nc.tensor.matmul(
                                        cp[:, :], vaug[:, ki, ds(65 * hh, 65)],
                                        et[:, ds(hb, 512)],
                                        start=(ki == 0), stop=False)
                                ctxq.append(ctx_ki)
                                pump_ctx(TRAIL)
                                tick()
                        else:
                            # diagonal pair: pack trimmed regions
                            # contiguously so one exp covers both
                            a0 = k0 - (nk - 4)
                            lo0, lo1 = a0 * 128, (a0 + 1) * 128
                            w0, w1 = 512 - lo0, 512 - lo1
                            offs = ((k0, lo0, w0, 0), (k1, lo1, w1, w0))
                            for ki, lo, w, off in offs:
                                nc.tensor.matmul(
                                    sp[:, ds(off, w)],
                                    kt[ds(64 * hh, 64), ts(ki, 128)].opt(),
                                    qt[ds(64 * hh, 64),
                                       ds(qi * 512 + lo, w)].opt(),
                                    start=True, stop=True)
                            nc.scalar.activation(et[:, ds(0, w0 + w1)],
                                                 sp[:, ds(0, w0 + w1)],
                                                 EXP, scale=0.125)
                            for ki, lo, w, off in offs:
                                # zero upper triangle of the diagonal band:
                                # et[k, z] = 0 where z < k
                                nc.gpsimd.affine_select(
                                    out=et[:, ds(off, 128)],
                                    in_=et[:, ds(off, 128)],
                                    compare_op=mybir.AluOpType.is_ge,
                                    fill=0.0, base=0, pattern=[[1, 128]],
                                    channel_multiplier=-1)
                            for ki, lo, w, off in offs:

                                def ctx_ki(ki=ki, lo=lo, w=w, off=off, et=et,
                                           cp=cp, hh=hh, vaug=vaug):
                                    st = ki == 0
                                    vsl = vaug[:, ki, ds(65 * hh, 65)]
                                    nc.tensor.matmul(cp[:, ds(lo, 128)], vsl,
                                                     et[:, ds(off, 128)],
                                                     start=st, stop=True)
                                    if w > 128:
                                        nc.tensor.matmul(
                                            cp[:, ds(lo + 128, w - 128)], vsl,
                                            et[:, ds(off + 128, w - 128)],
                                            start=st, stop=False)
                                ctxq.append(ctx_ki)
                                pump_ctx(TRAIL)
                                tick()
                    # stage epilogue, deferred: A = 1/denom on DVE;
                    # B = broadcast matmul + normalize-mul + outproj enqueue
                    cell = {}

                    def partA(cp=cp, cell=cell):
                        # 1/denom = exp(-ln(denom)) on ACT
                        lg = ep_sb.tile([1, 512], F32, tag="lg", name="lg")
                        nc.scalar.activation(lg[:], cp[64:65, :], LN)
                        rr = ep_sb.tile([1, 512], BF16, tag="rr", name="rr")
                        nc.scalar.activation(rr[:], lg[:], EXP, scale=-1.0)
                        cell["rr"] = rr

                    def partB(qi=qi, hh=hh, cp=cp, cell=cell, ctxT=ctxT, b=b):
                        rr = cell["rr"]
                        bcp = qkv_ps.tile([64, 512], F32, tag="qkv", name="bcp")
                        nc.tensor.matmul(bcp[:], ones_lr[:], rr[:],
                                         start=True, stop=True)
                        bcs = ep_sb.tile([64, 512], F32, tag="bcs", name="bcs")
                        nc.vector.tensor_copy(bcs[:], bcp[:])
                        nc.vector.tensor_mul(
                            ctxT[ds(64 * hh, 64), ts(qi, 512)],
                            cp[0:64, :], bcs[:])
                        if hh == 1:
                            rdy = state["slot"] + 4
                            for c in range(4):
                                for nn in range(2):
                                    aux.append(
                                        (rdy, make_outproj(b, qi, c, nn, ctxT)))

                    ctxq.append(partA)
                    if state["partB"] is not None:
                        ctxq.append(state["partB"])
                    state["partB"] = partB
                if qi == 1:
                    emit_x_dma(b + 1, 1)

        # ---- main ----
        emit_x_dma(0, 0, split=True)
        nc.sync.dma_start(wk_sb[:], wk.rearrange("(j p) h -> p j h", p=128))
        nc.sync.dma_start(wv_sb[:], wv.rearrange("(j p) h -> p j h", p=128))
        emit_x_dma(0, 1)
        nc.sync.dma_start(wo_sb[:], wo[:, :])
        for b in range(B):
            qt = qk_sb.tile([128, S], BF16, tag="qt", name="qt")
            kt = qk_sb.tile([128, S], BF16, tag="kt", name="kt")
            vaug = vpool.tile([128, NKC, 130], BF16, name="vaug")
            ctxT = ctx_sb.tile([128, S], BF16, tag="ctx", name="ctxT")
            prev = qkv_batch(b, qt, kt, vaug)
            emit_x_dma(b + 1, 0)
            emit_transposes(prev[0], prev[1], vaug)()
            attn_batch(b, qt, kt, vaug, ctxT)
        # flush
        pump_ctx(0)
        if state["partB"] is not None:
            state["partB"]()
            state["partB"] = None
        pump_ctx(0)
        while aux:
            pump_aux(1, force=True)

    return nc


# ======== host-side wrapper ========
_CACHE = {}


def _get_program():
    if "nc" not in _CACHE:
        install()
        _CACHE["nc"] = build()
    return _CACHE["nc"]


def _run(inputs, trace=False):
    import ml_dtypes
    from concourse.bass_utils import run_bass_kernel_spmd

    bf16 = ml_dtypes.bfloat16
    x = np.asarray(inputs["x"], dtype=np.float32)
    WQ = np.asarray(inputs["WQ"], dtype=np.float32)
    WK = np.asarray(inputs["WK"], dtype=np.float32)
    WV = np.asarray(inputs["WV"], dtype=np.float32)
    WO = np.asarray(inputs["WO"], dtype=np.float32)

    xTh = np.ascontiguousarray(x.reshape(BS, D).T.astype(bf16))
    woT = WO.T.astype(bf16)
    in_maps = []
    for c in range(NC):
        sl = slice(c * 128, (c + 1) * 128)
        in_maps.append({
            "xT": xTh,
            "wq": np.ascontiguousarray(WQ[sl, :].T.astype(bf16)),
            "wk": np.ascontiguousarray(WK[sl, :].T.astype(bf16)),
            "wv": np.ascontiguousarray(WV[sl, :].T.astype(bf16)),
            "wo": np.ascontiguousarray(woT[sl, :]),
        })

    nc_prog = _get_program()
    res = run_bass_kernel_spmd(nc_prog, in_maps, list(range(NC)), trace=trace)

    actual = np.zeros((BS, D), dtype=np.float32)
    for c in range(NC):
        actual += np.asarray(res.results[c]["out"], dtype=np.float32)
    return actual.reshape(x.shape), res


def kernel(**inputs):
    out, _ = _run(inputs, trace=False)
    return out


# revision 54
# speedup vs baseline: 1.0561x; 1.0155x over previous
"""Self-contained Trainium2 kernel for nn_MultiHeadAttention_91070486544496.

B=4, S=2048, D=1024, H=16 causal MHA. 8-core SPMD, head-parallel:
each core computes QKV+attention for its 2 heads over all batches, then
multiplies its context slice by the matching WO rows and writes a full
[BS, D] partial output; the host sums the 8 partials. No collectives.

Schedule keeps the TensorEngine gap-free (DVFS ramps 1.2->2.4GHz after
3us of continuous busy): ctx matmuls trail the score stream via a
deferred-work queue, V-transposes are deferred one chunk, softmax
normalization (reciprocal on DVE, broadcast matmul, multiply on Pool)
trails by a full head-stage, and output-projection matmuls are injected
as PE filler between score pairs. Causal masking is done by trimming
fully-masked column bands (never computed) and zeroing the partially
masked 128-wide diagonal band of exp(scores) with a Pool affine_select.
"""
import sys

for _p in ("/opt/trn_rl_repo", "/root/.axon_site/_ro/trn_rl_repo"):
    if _p not in sys.path:
        sys.path.append(_p)

import numpy as np

# ======== runtime infra (axon NTFF hook, BIR wait splitter) ========

import contextlib
import ctypes
import json
import types

_SO_PATH = "/opt/axon/libaxon_pjrt.so"


def _ntff_profile_via_ctypes(so_path):
    lib = ctypes.CDLL(so_path)
    if not hasattr(lib, "axon_start_nrt_profile"):
        return None
    lib.axon_start_nrt_profile.argtypes = [
        ctypes.POINTER(ctypes.c_int64),
        ctypes.c_size_t,
    ]
    lib.axon_start_nrt_profile.restype = ctypes.c_int64
    lib.axon_stop_nrt_profile.argtypes = [ctypes.c_char_p]
    lib.axon_stop_nrt_profile.restype = ctypes.c_int64

    @contextlib.contextmanager
    def _hook(output_dir, device_ids):
        import jax
        jax.devices()
        if device_ids:
            ids = (ctypes.c_int64 * len(device_ids))(*device_ids)
            rc = lib.axon_start_nrt_profile(ids, len(device_ids))
        else:
            rc = lib.axon_start_nrt_profile(None, 0)
        if rc != 0:
            raise RuntimeError(f"axon_start_nrt_profile rc={rc}")
        try:
            yield
        finally:
            n = lib.axon_stop_nrt_profile(str(output_dir).encode())
            if n < 0:
                raise RuntimeError(f"axon_stop_nrt_profile rc={n}")

    return _hook


def split_multi_waits(bir_json: bytes) -> bytes:
    d = json.loads(bir_json)
    n_split = 0
    for fn in d.get("functions", []):
        for blk in fn.get("blocks", []):
            insts = blk.get("instructions", [])
            out = []
            for inst in insts:
                si = inst.get("sync_info")
                waits = (si or {}).get("on_wait") or []
                if len(waits) > 1:
                    extra, keep = waits[:-1], waits[-1:]
                    for k, w in enumerate(extra):
                        out.append({
                            "debug": inst.get("debug", 0),
                            "engine": inst["engine"],
                            "ins": [],
                            "outs": [],
                            "name": f"{inst['name']}-ws{k}",
                            "opcode": "NoOp",
                            "sync_info": {"on_update": [], "on_wait": [w]},
                        })
                        n_split += 1
                    si["on_wait"] = keep
                out.append(inst)
            blk["instructions"] = out
    if n_split:
        print(f"bass_infra: split {n_split} extra sync waits into NoOps")
    return json.dumps(d).encode()


def install():
    # 1. antenv.axon_hooks shim
    if "antenv.axon_hooks" not in sys.modules:
        mod = types.ModuleType("antenv.axon_hooks")
        _state = {"hook": _ntff_profile_via_ctypes(_SO_PATH)}
        mod.set_axon_ntff_profile_hook = lambda h: _state.__setitem__("hook", h)
        mod.get_axon_ntff_profile_hook = lambda: _state["hook"]
        sys.modules["antenv.axon_hooks"] = mod
        import antenv
        antenv.axon_hooks = mod

    from concourse import bass_utils, bass2jax

    # 2. upload_artifacts stub
    bass_utils.upload_artifacts = lambda tmpdir: tmpdir

    # 3. wait-splitting compile wrapper
    orig_compile = bass_utils.compile_bir_kernel

    def compile_with_split(bir_json, tmpdir, neff_name="file.neff"):
        return orig_compile(split_multi_waits(bir_json), tmpdir, neff_name=neff_name)

    if getattr(bass2jax.compile_bir_kernel, "__name__", "") != "compile_with_split":
        bass_utils.compile_bir_kernel = compile_with_split
        bass2jax.compile_bir_kernel = compile_with_split


# ======== kernel IR builder ========
from collections import deque
from contextlib import ExitStack

import concourse.bass as bass
import concourse.mybir as mybir
import concourse.tile as tile
from concourse.bass import ds, ts
from concourse.masks import make_identity

F32 = mybir.dt.float32
BF16 = mybir.dt.bfloat16
EXP = mybir.ActivationFunctionType.Exp
LN = mybir.ActivationFunctionType.Ln

B, S, D, H, DK = 4, 2048, 1024, 16, 64
NC = 8          # cores
HL = 2          # heads per core
BS = B * S      # 8192
NQ = S // 512   # q-chunks per batch = 4
NKC = S // 128  # k-chunks per batch = 16
NDC = D // 128  # d_in chunks = 8
TRAIL = 8       # deferred-work queue depth (ki items)


def build(cfg=None):
    nc = bass.Bass("TRN2", target_bir_lowering=False, debug=False, num_devices=NC)

    xT = nc.dram_tensor("xT", [D, BS], BF16, kind="ExternalInput")
    wq = nc.dram_tensor("wq", [D, 128], BF16, kind="ExternalInput")
    wk = nc.dram_tensor("wk", [D, 128], BF16, kind="ExternalInput")
    wv = nc.dram_tensor("wv", [D, 128], BF16, kind="ExternalInput")
    wo = nc.dram_tensor("wo", [128, D], BF16, kind="ExternalInput")
    out = nc.dram_tensor("out", [BS, D], BF16, kind="ExternalOutput")

    with tile.TileContext(nc) as tc, ExitStack() as ctx:
        const = ctx.enter_context(tc.tile_pool(name="const", bufs=1))
        wpool = ctx.enter_context(tc.tile_pool(name="wpool", bufs=1))
        xpool = ctx.enter_context(tc.tile_pool(name="xpool", bufs=3))
        qkv_ps = ctx.enter_context(tc.tile_pool(name="qkv_ps", bufs=2, space="PSUM"))
        sp_ps = ctx.enter_context(tc.tile_pool(name="sp_ps", bufs=2, space="PSUM"))
        cp_ps = ctx.enter_context(tc.tile_pool(name="cp_ps", bufs=2, space="PSUM"))
        qk_sb = ctx.enter_context(tc.tile_pool(name="qk_sb", bufs=1))
        vpool = ctx.enter_context(tc.tile_pool(name="vpool", bufs=2))
        et_sb = ctx.enter_context(tc.tile_pool(name="et_sb", bufs=12))
        ep_sb = ctx.enter_context(tc.tile_pool(name="ep_sb", bufs=2))
        ctx_sb = ctx.enter_context(tc.tile_pool(name="ctx_sb", bufs=2))
        ob_sb = ctx.enter_context(tc.tile_pool(name="ob_sb", bufs=4))

        # ---- constants ----
        ident = const.tile([128, 128], F32)
        make_identity(nc, ident[:])
        identr = const.tile([128, 128], BF16)
        nc.vector.tensor_copy(identr[:], ident[:])
        onesf = const.tile([128, 16], F32)
        nc.vector.memset(onesf[:], 1.0)
        ones_l = const.tile([1, 64], F32)
        nc.vector.memset(ones_l[:], 1.0)
        ones_lr = const.tile([1, 64], BF16)
        nc.vector.tensor_copy(ones_lr[:], ones_l[:])
        ones_row = const.tile([1, 512], F32)
        nc.vector.memset(ones_row[:], 1.0)

        # ---- weights ----
        wq_sb = wpool.tile([128, NDC, 128], BF16)
        wk_sb = wpool.tile([128, NDC, 128], BF16)
        wv_sb = wpool.tile([128, NDC, 128], BF16)
        wo_sb = wpool.tile([128, D], BF16)
        nc.sync.dma_start(wq_sb[:], wq.rearrange("(j p) h -> p j h", p=128))

        # ---- deferred work ----
        ctxq = deque()   # trailing ctx matmuls + softmax epilogue parts
        aux = deque()    # out-projection bundles (PE filler)
        state = {"partB": None, "slot": 0}

        def pump_ctx(target):
            while len(ctxq) > target:
                ctxq.popleft()()

        def pump_aux(n=1, force=False):
            for _ in range(n):
                if not aux:
                    return
                if not force and aux[0][0] > state["slot"]:
                    return
                aux.popleft()[1]()

        def tick():
            state["slot"] += 1
            if state["slot"] % 2 == 0:
                pump_aux(1)

        # ---- x prefetch ----
        xt_tiles = {}

        def emit_x_dma(b, i, split=False):
            if b >= B or (b, i) in xt_tiles:
                return
            t = xpool.tile([128, NDC, 512], BF16, tag="xt", name=f"xt{b}_{i}")
            xt_tiles[(b, i)] = t
            src = xT.rearrange("(j p) n -> p j n", p=128)[:, :, ds(b * S + i * 512, 512)]
            if split:
                for j in range(NDC):
                    nc.sync.dma_start(t[:, j, :], src[:, j, :])
            else:
                nc.sync.dma_start(t[:], src)

        # ---- out-projection bundle: one PE matmul + Pool copy + DMA ----
        def make_outproj(b, qi, c, nn, ctxT):
            def run():
                op = qkv_ps.tile([128, 512], F32, tag="qkv", name="op")
                nc.tensor.matmul(op[:], ctxT[:, ds(qi * 512 + c * 128, 128)],
                                 wo_sb[:, ds(nn * 512, 512)], start=True, stop=True)
                ob = ob_sb.tile([128, 512], BF16, tag="ob", name="ob")
                nc.vector.tensor_copy(ob[:], op[:])
                nc.sync.dma_start(
                    out[ds(b * S + qi * 512 + c * 128, 128), ds(nn * 512, 512)],
                    ob[:])
            return run

        # ---- QKV ----
        def emit_transposes(vs, i, vaug):
            # 4 transposes into one psum tile, then 2 strided vaug copies
            vtp = qkv_ps.tile([128, 512], BF16, tag="qkv", name="vtp")
            for j4 in range(4):
                nc.tensor.transpose(vtp[:, ts(j4, 128)], vs[:, ts(j4, 128)],
                                    identr[:])

            def copies():
                v3d = vtp.rearrange("p (j c) -> p j c", j=4)
                nc.vector.tensor_copy(vaug[:, ds(4 * i, 4), 0:64],
                                      v3d[:, :, 0:64])
                nc.vector.tensor_copy(vaug[:, ds(4 * i, 4), 65:129],
                                      v3d[:, :, 64:128])
            return copies

        def qkv_batch(b, qt, kt, vaug):
            nc.vector.tensor_copy(vaug[:, :, 64:65].opt(), onesf[:, 0:NKC])
            nc.vector.tensor_copy(vaug[:, :, 129:130].opt(), onesf[:, 0:NKC])
            prev = None
            for i in range(NQ):
                emit_x_dma(b, i)
                xt = xt_tiles.pop((b, i))
                qp = qkv_ps.tile([128, 512], F32, tag="qkv", name="qp")
                for j in range(NDC):
                    nc.tensor.matmul(qp[:], wq_sb[:, j, :], xt[:, j, :],
                                     start=(j == 0), stop=(j == NDC - 1))
                pump_ctx(max(0, 4 - 2 * i))
                pump_aux(1)
                kp = qkv_ps.tile([128, 512], F32, tag="qkv", name="kp")
                for j in range(NDC):
                    nc.tensor.matmul(kp[:], wk_sb[:, j, :], xt[:, j, :],
                                     start=(j == 0), stop=(j == NDC - 1))
                nc.vector.tensor_copy(qt[:, ts(i, 512)], qp[:])
                nc.vector.tensor_copy(kt[:, ts(i, 512)], kp[:])
                pump_ctx(max(0, 3 - 2 * i))
                vcopies = None
                if prev is not None:
                    vcopies = emit_transposes(prev[0], prev[1], vaug)
                pump_aux(1)
                vp = qkv_ps.tile([128, 512], F32, tag="qkv", name="vp")
                for j in range(NDC):
                    nc.tensor.matmul(vp[:], wv_sb[:, j, :], xt[:, j, :],
                                     start=(j == 0), stop=(j == NDC - 1))
                vs = ep_sb.tile([128, 512], BF16, tag="vs", name="vs")
                nc.vector.tensor_copy(vs[:], vp[:])
                if vcopies is not None:
                    vcopies()
                prev = (vs, i)
                pump_ctx(max(0, 2 - 2 * i))
                pump_aux(1)
            return prev

        # ---- attention ----
        def attn_batch(b, qt, kt, vaug, ctxT):
            for qi in range(NQ):
                nk = 4 * qi + 4
                for hh in range(HL):
                    cp = cp_ps.tile([65, 512], F32, tag="cp", name="cp")
                    for p in range(nk // 2):
                        sp = sp_ps.tile([128, 1024], F32, tag="sp", name="sp")
                        et = et_sb.tile([128, 1024], BF16, tag="et", name="et")
                        k0, k1 = 2 * p, 2 * p + 1
                        if k0 < nk - 4:
                            # non-diagonal pair: full-width halves, one exp
                            for ki in (k0, k1):
                                hb = 512 * (ki & 1)
                                nc.tensor.matmul(
                                    sp[:, ds(hb, 512)],
                                    kt[ds(64 * hh, 64), ts(ki, 128)].opt(),
                                    qt[ds(64 * hh, 64), ts(qi, 512)].opt(),
                                    start=True, stop=True)
                            nc.scalar.activation(et[:], sp[:], EXP, scale=0.125)
                            for ki in (k0, k1):
                                hb = 512 * (ki & 1)

                                def ctx_ki(ki=ki, hb=hb, et=et, cp=cp, hh=hh,
                                           vaug=vaug):
                                    nc.tensor.matmul(
                                        cp[:, :], vaug[:, ki, ds(65 * hh, 65)],
                                        et[:, ds(hb, 512)],
                                        start=(ki == 0), stop=False)
                                ctxq.append(ctx_ki)
                                pump_ctx(TRAIL)
                                tick()
                        else:
                            # diagonal pair: pack trimmed regions
                            # contiguously so one exp covers both
                            a0 = k0 - (nk - 4)
                            lo0, lo1 = a0 * 128, (a0 + 1) * 128
                            w0, w1 = 512 - lo0, 512 - lo1
                            offs = ((k0, lo0, w0, 0), (k1, lo1, w1, w0))
                            for ki, lo, w, off in offs:
                                nc.tensor.matmul(
                                    sp[:, ds(off, w)],
                                    kt[ds(64 * hh, 64), ts(ki, 128)].opt(),
                                    qt[ds(64 * hh, 64),
                                       ds(qi * 512 + lo, w)].opt(),
                                    start=True, stop=True)
                            nc.scalar.activation(et[:, ds(0, w0 + w1)],
                                                 sp[:, ds(0, w0 + w1)],
                                                 EXP, scale=0.125)
                            for ki, lo, w, off in offs:
                                # zero upper triangle of the diagonal band:
                                # et[k, z] = 0 where z < k
                                nc.gpsimd.affine_select(
                                    out=et[:, ds(off, 128)],
                                    in_=et[:, ds(off, 128)],
                                    compare_op=mybir.AluOpType.is_ge,
                                    fill=0.0, base=0, pattern=[[1, 128]],
                                    channel_multiplier=-1)
                            for ki, lo, w, off in offs:

                                def ctx_ki(ki=ki, lo=lo, w=w, off=off, et=et,
                                           cp=cp, hh=hh, vaug=vaug):
                                    st = ki == 0
                                    vsl = vaug[:, ki, ds(65 * hh, 65)]
                                    nc.tensor.matmul(cp[:, ds(lo, 128)], vsl,
                                                     et[:, ds(off, 128)],
                                                     start=st, stop=True)
                                    if w > 128:
                                        nc.tensor.matmul(
                                            cp[:, ds(lo + 128, w - 128)], vsl,
                                            et[:, ds(off + 128, w - 128)],
                                            start=st, stop=False)
                                ctxq.append(ctx_ki)
                                pump_ctx(TRAIL)
                                tick()
                    # stage epilogue, deferred: A = 1/denom on DVE;
                    # B = broadcast matmul + normalize-mul + outproj enqueue
                    cell = {}

                    def partA(cp=cp, cell=cell):
                        # 1/denom = exp(-ln(denom)) on ACT
                        lg = ep_sb.tile([1, 512], F32, tag="lg", name="lg")
                        nc.scalar.activation(lg[:], cp[64:65, :], LN)
                        rr = ep_sb.tile([1, 512], BF16, tag="rr", name="rr")
                        nc.scalar.activation(rr[:], lg[:], EXP, scale=-1.0)
                        cell["rr"] = rr

                    def partB(qi=qi, hh=hh, cp=cp, cell=cell, ctxT=ctxT, b=b):
                        rr = cell["rr"]
                        bcp = qkv_ps.tile([64, 512], F32, tag="qkv", name="bcp")
                        nc.tensor.matmul(bcp[:], ones_lr[:], rr[:],
                                         start=True, stop=True)
                        bcs = ep_sb.tile([64, 512], F32, tag="bcs", name="bcs")
                        nc.vector.tensor_copy(bcs[:], bcp[:])
                        nc.vector.tensor_mul(
                            ctxT[ds(64 * hh, 64), ts(qi, 512)],
                            cp[0:64, :], bcs[:])
                        if hh == 1:
                            rdy = state["slot"] + 4
                            for c in range(4):
                                for nn in range(2):
                                    aux.append(
                                        (rdy, make_outproj(b, qi, c, nn, ctxT)))

                    ctxq.append(partA)
                    if state["partB"] is not None:
                        ctxq.append(state["partB"])
                    state["partB"] = partB
                if qi == 1:
                    emit_x_dma(b + 1, 1)

        # ---- main ----
        emit_x_dma(0, 0, split=True)
        nc.sync.dma_start(wk_sb[:], wk.rearrange("(j p) h -> p j h", p=128))
        nc.sync.dma_start(wv_sb[:], wv.rearrange("(j p) h -> p j h", p=128))
        emit_x_dma(0, 1)
        nc.sync.dma_start(wo_sb[:], wo[:, :])
        for b in range(B):
            qt = qk_sb.tile([128, S], BF16, tag="qt", name="qt")
            kt = qk_sb.tile([128, S], BF16, tag="kt", name="kt")
            vaug = vpool.tile([128, NKC, 130], BF16, name="vaug")
            ctxT = ctx_sb.tile([128, S], BF16, tag="ctx", name="ctxT")
            prev = qkv_batch(b, qt, kt, vaug)
            emit_x_dma(b + 1, 0)
            emit_transposes(prev[0], prev[1], vaug)()
            attn_batch(b, qt, kt, vaug, ctxT)
        # flush
        pump_ctx(0)
        if state["partB"] is not None:
            state["partB"]()
            state["partB"] = None
        pump_ctx(0)
        while aux:
            pump_aux(1, force=True)

    return nc


# ======== host-side wrapper ========
_CACHE = {}


def _get_program():
    if "nc" not in _CACHE:
        install()
        _CACHE["nc"] = build()
    return _CACHE["nc"]


def _run(inputs, trace=False):
    import ml_dtypes
    from concourse.bass_utils import run_bass_kernel_spmd

    bf16 = ml_dtypes.bfloat16
    x = np.asarray(inputs["x"], dtype=np.float32)
    WQ = np.asarray(inputs["WQ"], dtype=np.float32)
    WK = np.asarray(inputs["WK"], dtype=np.float32)
    WV = np.asarray(inputs["WV"], dtype=np.float32)
    WO = np.asarray(inputs["WO"], dtype=np.float32)

    xTh = np.ascontiguousarray(x.reshape(BS, D).T.astype(bf16))
    woT = WO.T.astype(bf16)
    in_maps = []
    for c in range(NC):
        sl = slice(c * 128, (c + 1) * 128)
        in_maps.append({
            "xT": xTh,
            "wq": np.ascontiguousarray(WQ[sl, :].T.astype(bf16)),
            "wk": np.ascontiguousarray(WK[sl, :].T.astype(bf16)),
            "wv": np.ascontiguousarray(WV[sl, :].T.astype(bf16)),
            "wo": np.ascontiguousarray(woT[sl, :]),
        })

    nc_prog = _get_program()
    res = run_bass_kernel_spmd(nc_prog, in_maps, list(range(NC)), trace=trace)

    actual = np.zeros((BS, D), dtype=np.float32)
    for c in range(NC):
        actual += np.asarray(res.results[c]["out"], dtype=np.float32)
    return actual.reshape(x.shape), res


def kernel(**inputs):
    out, _ = _run(inputs, trace=False)
    return out
